# revision 14
# baseline (speedup 1.0000x reference)
"""CenterLoss kernel for Trainium2 (Bass/Tile), 8-core data-parallel.

loss = sum_i ||x_i - centers[labels_i]||^2
  x: (65536, 512) f32, labels: (65536,) int, centers: (512, 512) f32

Per-core plan (8192 rows each), using the expansion
  loss = sum x^2 - 2*sum_{c,d} S[c,d]*centers[c,d] + sum_c count_c*||C_c||^2
with S = onehot(labels)^T @ x computed on the PE via one-hot matmuls.
Label/center prep (one-hot iota, label layout, histogram * row-norms) is
host-side input formatting; all O(B*D) work runs on device.

v3: p-major x mapping (row r -> partition r//64, slot r%64) so the f32->fp8
casting SWDGE stream uses 16KB-contiguous HBM reads per descriptor, 4KB-write
packets.  S accumulates into two PSUM groups so the S*C contraction for the
first group overlaps the tail of streaming; sum(x^2) runs on ACT per
supertile; no other work competes with the stream.
"""

import sys

import numpy as np

sys.path.insert(0, "/opt/trn_rl_repo")

N_CORES = 8
B = 65536
D = 512
B_L = B // N_CORES  # 8192 rows per core
Q = 64  # rows per partition (p-major: row = p*Q + t)
SUP = 8  # rows-per-partition per supertile DMA
N_SUP = Q // SUP  # 8
N_PAIRS = Q // 2  # 32 DoubleRow pairs
NCH = D // 128  # 4 class chunks
SPLIT_PAIR = 30  # pairs [0,30) -> S_a, [30,32) -> S_b

_CACHE = {}


def _build():
    """Trace the Bass/Tile program once; returns the compiled Bacc module."""
    if "nc" in _CACHE:
        return _CACHE["nc"]

    import concourse.bacc as bacc
    import concourse.mybir as mybir
    import concourse.tile as tile

    f32 = mybir.dt.float32
    bf16 = mybir.dt.bfloat16
    fp8 = mybir.dt.float8e4

    nc = bacc.Bacc("TRN2", debug=False, num_devices=N_CORES)
    x_t = nc.dram_tensor("x", [B_L, D], f32, kind="ExternalInput")
    iota_t = nc.dram_tensor("iota16", [128, D], mybir.dt.float16, kind="ExternalInput")
    labf_t = nc.dram_tensor("labf", [128, Q], f32, kind="ExternalInput")
    cc_t = nc.dram_tensor("cntcsq", [128, NCH], f32, kind="ExternalInput")
    c_t = nc.dram_tensor("centers", [D, D], f32, kind="ExternalInput")
    out_t = nc.dram_tensor("out", [128, 1], f32, kind="ExternalOutput")

    with tile.TileContext(nc) as tc:
        with (
            tc.tile_pool(name="io", bufs=5) as io_pool,
            tc.tile_pool(name="oh", bufs=12) as oh_pool,
            tc.tile_pool(name="psum", bufs=1, space="PSUM") as psum_pool,
            tc.tile_pool(name="misc", bufs=1) as misc_pool,
        ):
            iota_sb = misc_pool.tile([128, D], mybir.dt.float16)
            nc.sync.dma_start(iota_sb[:], iota_t.ap())
            labf_sb = misc_pool.tile([128, Q], f32)
            nc.sync.dma_start(labf_sb[:], labf_t.ap())
            cent_sb = misc_pool.tile([128, NCH, D], f32)
            nc.sync.dma_start(
                cent_sb[:], c_t.ap().rearrange("(n p) d -> p n d", p=128)
            )

            # fin: one accumulator column per partial term; a single reduce
            # at the end produces the per-core partial sums.
            #   cols [0, 10): sum(x^2) per supertile-chunk (ACT)
            #   cols [10, 18): -2*S*C per chunk/group (DVE)
            #   cols [18, 22): host-side count*csq (DMA)
            N_SQ = N_SUP + 2
            fin = misc_pool.tile([128, N_SQ + 2 * NCH + NCH], f32)
            nc.sync.dma_start(fin[:, N_SQ + 2 * NCH :], cc_t.ap())
            junk_dve = misc_pool.tile([128, 1], f32)
            junk_act = misc_pool.tile([128, 1], f32)
            r1 = misc_pool.tile([128, 1], f32)

            S_all = psum_pool.tile([128, NCH, D], f32, name="S_all")

            def s_dot_c(S, c, col):
                # fin[:, N_SQ+col] = -2 * sum_d S[c, d] * C[c, d]
                col += N_SQ
                nc.vector.scalar_tensor_tensor(
                    out=junk_dve[:].broadcast_to(S[:, c, :].shape),
                    in0=S[:, c, :],
                    scalar=-2.0,
                    in1=cent_sb[:, c, :],
                    op0=mybir.AluOpType.mult,
                    op1=mybir.AluOpType.mult,
                    accum_out=fin[:, col : col + 1],
                )

            def sq(x_ap, col):
                # fin[:, col] = sum(x_ap^2) on ACT
                nc.scalar.activation(
                    junk_act[:].broadcast_to(x_ap.shape),
                    x_ap,
                    mybir.ActivationFunctionType.Square,
                    accum_out=fin[:, col : col + 1],
                )

            x_re = x_t.ap().rearrange("(p q) d -> p q d", p=128)
            for s in range(N_SUP):
                x_sb = io_pool.tile([128, SUP, D], fp8, tag="x")
                lo = SUP * s
                # SWDGE casts f32 -> fp8e4m3 in flight; p-major layout makes
                # each descriptor a 16KB contiguous HBM read.  First supertile
                # split per pair so the PE pipeline starts as early as
                # possible; last supertile split so the tail is short.
                if s == 0:
                    for k in range(SUP // 2):
                        nc.gpsimd.dma_start(
                            x_sb[:, 2 * k : 2 * k + 2, :],
                            x_re[:, lo + 2 * k : lo + 2 * k + 2, :],
                        )
                elif s == N_SUP - 1:
                    for a, b in ((0, 4), (4, 6), (6, 8)):
                        nc.gpsimd.dma_start(
                            x_sb[:, a:b, :], x_re[:, lo + a : lo + b, :]
                        )
                else:
                    nc.gpsimd.dma_start(x_sb[:], x_re[:, lo : lo + SUP, :])
                for jj in range(SUP // 2):
                    j = (SUP // 2) * s + jj  # global pair index
                    oh = oh_pool.tile([128, 2, D], fp8, tag="oh")
                    for u in range(2):
                        t = lo + 2 * jj + u
                        nc.vector.tensor_scalar(
                            out=oh[:, u, :],
                            in0=iota_sb[:],
                            scalar1=labf_sb[:, t : t + 1],
                            scalar2=None,
                            op0=mybir.AluOpType.is_equal,
                        )
                    first = j == 0
                    last = j == N_PAIRS - 1
                    for c in range(NCH):
                        nc.tensor.matmul(
                            S_all[:, c, :],
                            lhsT=oh[:, :, c * 128 : (c + 1) * 128],
                            rhs=x_sb[:, 2 * jj : 2 * jj + 2, :],
                            start=first,
                            stop=last,
                            perf_mode=mybir.MatmulPerfMode.DoubleRow,
                        )
                # sum(x^2): one ACT op per supertile; last supertile split to
                # match its sub-DMAs so the tail stays short
                if s == N_SUP - 1:
                    for k, (a, b) in enumerate(((0, 4), (4, 6), (6, 8))):
                        sq(
                            x_sb[:, a:b, :].rearrange("p q d -> p (q d)"),
                            s + k,
                        )
                else:
                    sq(x_sb[:].rearrange("p q d -> p (q d)"), s)

            # tail: chunk c's contraction starts as soon as its stop matmul
            # retires, pipelining with the remaining chunks' matmuls
            for c in range(NCH):
                s_dot_c(S_all, c, c)
            nc.vector.tensor_reduce(
                r1[:], fin[:], axis=mybir.AxisListType.X, op=mybir.AluOpType.add
            )
            nc.gpsimd.dma_start(out_t.ap(), r1[:])

    nc.compile()
    _CACHE["nc"] = nc
    return nc


def _prep_inputs(x, labels, centers):
    """Shard full inputs into the 8 per-core input maps."""
    x = np.asarray(x, dtype=np.float32)
    labels = np.asarray(labels)
    centers = np.ascontiguousarray(np.asarray(centers, dtype=np.float32))
    iota16 = np.ascontiguousarray(
        np.tile(np.arange(D, dtype=np.float16), (128, 1))
    )
    csq = (centers.astype(np.float64) ** 2).sum(axis=1)  # [D]
    in_maps = []
    for c in range(N_CORES):
        xs = np.ascontiguousarray(x[c * B_L : (c + 1) * B_L])
        lab = labels[c * B_L : (c + 1) * B_L]
        # p-major: row r = p*Q + t  ->  labf[p, t]
        labf = np.ascontiguousarray(lab.reshape(128, Q).astype(np.float32))
        cnt = np.bincount(lab, minlength=D).astype(np.float64)
        cntcsq = np.ascontiguousarray(
            (cnt * csq).reshape(NCH, 128).T.astype(np.float32)
        )
        in_maps.append(
            {
                "x": xs,
                "iota16": iota16,
                "labf": labf,
                "cntcsq": cntcsq,
                "centers": centers,
            }
        )
    return in_maps


def _run(x, labels, centers, trace=False):
    from concourse import bass_utils

    nc = _build()
    in_maps = _prep_inputs(x, labels, centers)
    res = bass_utils.run_bass_kernel_spmd(
        nc, in_maps, core_ids=list(range(N_CORES)), trace=trace
    )
    total = np.float64(0.0)
    for r in res.results:
        total += np.sum(r["out"].astype(np.float64))
    return np.array(total, dtype=np.float32), res


def kernel(x, labels, centers):
    out, _ = _run(x, labels, centers, trace=False)
    return out


def kernel_traced(x, labels, centers):
    return _run(x, labels, centers, trace=True)


# revision 15
# speedup vs baseline: 1.0916x; 1.0916x over previous
"""CenterLoss kernel for Trainium2 (Bass/Tile), 8-core data-parallel.

loss = sum_i ||x_i - centers[labels_i]||^2
  x: (65536, 512) f32, labels: (65536,) int, centers: (512, 512) f32

Per-core plan (8192 rows each), using the expansion
  loss = sum x^2 - 2*sum_{c,d} S[c,d]*centers[c,d] + sum_c count_c*||C_c||^2
with S = onehot(labels)^T @ x computed on the PE via one-hot matmuls.
Label/center prep (one-hot iota, label layout, histogram * row-norms) is
host-side input formatting; all O(B*D) work runs on device.

v3: p-major x mapping (row r -> partition r//64, slot r%64) so the f32->fp8
casting SWDGE stream uses 16KB-contiguous HBM reads per descriptor, 4KB-write
packets.  S accumulates into two PSUM groups so the S*C contraction for the
first group overlaps the tail of streaming; sum(x^2) runs on ACT per
supertile; no other work competes with the stream.
"""

import sys

import numpy as np

sys.path.insert(0, "/opt/trn_rl_repo")

N_CORES = 8
B = 65536
D = 512
B_L = B // N_CORES  # 8192 rows per core
Q = 64  # rows per partition (p-major: row = p*Q + t)
SUP = 8  # rows-per-partition per supertile DMA
N_SUP = Q // SUP  # 8
N_PAIRS = Q // 2  # 32 DoubleRow pairs
NCH = D // 128  # 4 class chunks
SPLIT_PAIR = 30  # pairs [0,30) -> S_a, [30,32) -> S_b

_CACHE = {}


def _build():
    """Trace the Bass/Tile program once; returns the compiled Bacc module."""
    if "nc" in _CACHE:
        return _CACHE["nc"]

    import concourse.bacc as bacc
    import concourse.mybir as mybir
    import concourse.tile as tile

    f32 = mybir.dt.float32
    bf16 = mybir.dt.bfloat16
    fp8 = mybir.dt.float8e4

    nc = bacc.Bacc("TRN2", debug=False, num_devices=N_CORES)
    x_t = nc.dram_tensor("x", [B_L, D], f32, kind="ExternalInput")
    iota_t = nc.dram_tensor("iota16", [128, D], mybir.dt.float16, kind="ExternalInput")
    labf_t = nc.dram_tensor("labf", [128, Q], f32, kind="ExternalInput")
    cc_t = nc.dram_tensor("cntcsq", [128, NCH], f32, kind="ExternalInput")
    c_t = nc.dram_tensor("centers", [D, D], f32, kind="ExternalInput")
    out_t = nc.dram_tensor("out", [128, 1], f32, kind="ExternalOutput")

    with tile.TileContext(nc) as tc:
        with (
            tc.tile_pool(name="io", bufs=5) as io_pool,
            tc.tile_pool(name="oh", bufs=12) as oh_pool,
            tc.tile_pool(name="psum", bufs=1, space="PSUM") as psum_pool,
            tc.tile_pool(name="misc", bufs=1) as misc_pool,
        ):
            iota_sb = misc_pool.tile([128, D], mybir.dt.float16)
            nc.sync.dma_start(iota_sb[:], iota_t.ap())
            labf_sb = misc_pool.tile([128, Q], f32)
            nc.sync.dma_start(labf_sb[:], labf_t.ap())
            cent_sb = misc_pool.tile([128, NCH, D], f32)
            nc.sync.dma_start(
                cent_sb[:], c_t.ap().rearrange("(n p) d -> p n d", p=128)
            )

            # fin: one accumulator column per partial term; a single reduce
            # at the end produces the per-core partial sums.
            #   cols [0, 10): sum(x^2) per supertile-chunk (ACT)
            #   cols [10, 14): -2*S*C per chunk (DVE)
            #   cols [14, 18): host-side count*csq (DMA)
            N_SQ = N_SUP + 2
            fin = misc_pool.tile([128, N_SQ + NCH + NCH], f32)
            nc.sync.dma_start(fin[:, N_SQ + NCH :], cc_t.ap())
            junk_dve = misc_pool.tile([128, 1], f32)
            junk_act = misc_pool.tile([128, 1], f32)
            r1 = misc_pool.tile([128, 1], f32)

            S_all = psum_pool.tile([128, NCH, D], f32, name="S_all")

            def s_dot_c(S, c, col):
                # fin[:, N_SQ+col] = -2 * sum_d S[c, d] * C[c, d]
                col += N_SQ
                nc.vector.scalar_tensor_tensor(
                    out=junk_dve[:].broadcast_to(S[:, c, :].shape),
                    in0=S[:, c, :],
                    scalar=-2.0,
                    in1=cent_sb[:, c, :],
                    op0=mybir.AluOpType.mult,
                    op1=mybir.AluOpType.mult,
                    accum_out=fin[:, col : col + 1],
                )

            def sq(x_ap, col):
                # fin[:, col] = sum(x_ap^2) on ACT
                nc.scalar.activation(
                    junk_act[:].broadcast_to(x_ap.shape),
                    x_ap,
                    mybir.ActivationFunctionType.Square,
                    accum_out=fin[:, col : col + 1],
                )

            x_re = x_t.ap().rearrange("(p q) d -> p q d", p=128)
            for s in range(N_SUP):
                x_sb = io_pool.tile([128, SUP, D], fp8, tag="x")
                lo = SUP * s
                # SWDGE casts f32 -> fp8e4m3 in flight; p-major layout makes
                # each descriptor a 16KB contiguous HBM read.  First supertile
                # split per pair so the PE pipeline starts as early as
                # possible; last supertile split so the tail is short.
                if s == 0:
                    for k in range(SUP // 2):
                        nc.gpsimd.dma_start(
                            x_sb[:, 2 * k : 2 * k + 2, :],
                            x_re[:, lo + 2 * k : lo + 2 * k + 2, :],
                        )
                elif s == N_SUP - 1:
                    for a, b in ((0, 4), (4, 6), (6, 8)):
                        nc.gpsimd.dma_start(
                            x_sb[:, a:b, :], x_re[:, lo + a : lo + b, :]
                        )
                else:
                    nc.gpsimd.dma_start(x_sb[:], x_re[:, lo : lo + SUP, :])
                for jj in range(SUP // 2):
                    j = (SUP // 2) * s + jj  # global pair index
                    oh = oh_pool.tile([128, 2, D], fp8, tag="oh")
                    for u in range(2):
                        t = lo + 2 * jj + u
                        nc.vector.tensor_scalar(
                            out=oh[:, u, :],
                            in0=iota_sb[:],
                            scalar1=labf_sb[:, t : t + 1],
                            scalar2=None,
                            op0=mybir.AluOpType.is_equal,
                        )
                    first = j == 0
                    last = j == N_PAIRS - 1
                    for c in range(NCH):
                        nc.tensor.matmul(
                            S_all[:, c, :],
                            lhsT=oh[:, :, c * 128 : (c + 1) * 128],
                            rhs=x_sb[:, 2 * jj : 2 * jj + 2, :],
                            start=first,
                            stop=last,
                            perf_mode=mybir.MatmulPerfMode.DoubleRow,
                        )
                # sum(x^2): one ACT op per supertile; last supertile split to
                # match its sub-DMAs so the tail stays short
                if s == N_SUP - 1:
                    for k, (a, b) in enumerate(((0, 4), (4, 6), (6, 8))):
                        sq(
                            x_sb[:, a:b, :].rearrange("p q d -> p (q d)"),
                            s + k,
                        )
                else:
                    sq(x_sb[:].rearrange("p q d -> p (q d)"), s)

            # tail: chunk c's contraction starts as soon as its stop matmul
            # retires, pipelining with the remaining chunks' matmuls
            for c in range(NCH):
                s_dot_c(S_all, c, c)
            nc.vector.tensor_reduce(
                r1[:], fin[:], axis=mybir.AxisListType.X, op=mybir.AluOpType.add
            )
            nc.gpsimd.dma_start(out_t.ap(), r1[:])

    nc.compile()
    _CACHE["nc"] = nc
    return nc


def _prep_inputs(x, labels, centers):
    """Shard full inputs into the 8 per-core input maps."""
    x = np.asarray(x, dtype=np.float32)
    labels = np.asarray(labels)
    centers = np.ascontiguousarray(np.asarray(centers, dtype=np.float32))
    iota16 = np.ascontiguousarray(
        np.tile(np.arange(D, dtype=np.float16), (128, 1))
    )
    csq = (centers.astype(np.float64) ** 2).sum(axis=1)  # [D]
    in_maps = []
    for c in range(N_CORES):
        xs = np.ascontiguousarray(x[c * B_L : (c + 1) * B_L])
        lab = labels[c * B_L : (c + 1) * B_L]
        # p-major: row r = p*Q + t  ->  labf[p, t]
        labf = np.ascontiguousarray(lab.reshape(128, Q).astype(np.float32))
        cnt = np.bincount(lab, minlength=D).astype(np.float64)
        cntcsq = np.ascontiguousarray(
            (cnt * csq).reshape(NCH, 128).T.astype(np.float32)
        )
        in_maps.append(
            {
                "x": xs,
                "iota16": iota16,
                "labf": labf,
                "cntcsq": cntcsq,
                "centers": centers,
            }
        )
    return in_maps


def _run(x, labels, centers, trace=False):
    from concourse import bass_utils

    nc = _build()
    in_maps = _prep_inputs(x, labels, centers)
    res = bass_utils.run_bass_kernel_spmd(
        nc, in_maps, core_ids=list(range(N_CORES)), trace=trace
    )
    total = np.float64(0.0)
    for r in res.results:
        total += np.sum(r["out"].astype(np.float64))
    return np.array(total, dtype=np.float32), res


def kernel(x, labels, centers):
    out, _ = _run(x, labels, centers, trace=False)
    return out


def kernel_traced(x, labels, centers):
    return _run(x, labels, centers, trace=True)
